# revision 5
# baseline (speedup 1.0000x reference)
"""Bass/Tile TRN2 kernel for nn_CenterAlignedTripletLoss (8-core SPMD).

Redesign v2 — single-collective feature sharding:
  Each core owns a 1024-wide feature slice of feats (host-staged in both
  natural [m, d] and transposed [d, m] bf16 layouts, so no on-chip DMA
  transposes).  It computes the center slice [96, 1024], per-sample
  partial norms, and partial mining scores s_c[n, m] = ||f_m||^2_c
  - 2 c_n . f_m over its slice.  ONE AllToAll exchanges, per destination
  core r, the partial scores for r's 12 centers plus this core's center
  slice of r's centers (packed [96, 1536+1024] f32).  Each core then
  sums the 8 partial-score blocks (PE fold), mines hardest pos/neg with
  host-staged label masks, indirect-gathers the 24 winning rows in
  stripe-major [96, 1024] layout, and computes the 8x8 stripe distance
  matrices via PE cross-products (instead of elementwise diff-square):
  d2 = ||x||^2 + ||y||^2 - 2 x.y accumulated in one [96, 192] psum.
  sqrt -> tanh(d/2) -> masked block-diag fold -> 9x9 shortest-path DP ->
  relu(ap - an + margin) partial sum.  Host sums the 8 scalars / 96.
"""

import numpy as np
from contextlib import ExitStack

import concourse.bass as bass
import concourse.bacc as bacc
import concourse.tile as tile
from concourse import mybir
from concourse import bass_utils

F32 = mybir.dt.float32
BF16 = mybir.dt.bfloat16
U32 = mybir.dt.uint32
AF = mybir.ActivationFunctionType
ALU = mybir.AluOpType

NCORES = 8
M = 1536          # samples
D = 8192          # feature dim
DPC = D // NCORES # 1024 features per core
N = 96            # centers
NB = N // NCORES  # 12 centers per core
K = 16            # samples per chunk
S = 8             # stripes
DL = 1024         # local feature dim per stripe
MARGIN = 0.3
BIGM = 1.0e9
BIGDP = 1.0e6
RG = [list(range(NCORES))]
NT = M // 128     # 12 natural m-tiles
PK = 1536 + DPC   # packed a2a row width (scores + center slice)


def build_body(tc, out, ins):
    nc = tc.nc

    with ExitStack() as ctx:
        const = ctx.enter_context(tc.tile_pool(name="const", bufs=1))
        pers = ctx.enter_context(tc.tile_pool(name="pers", bufs=1))
        dram = ctx.enter_context(tc.tile_pool(name="dram", bufs=1, space="DRAM"))

        # ---- constants ----
        ident = const.tile([128, 128], F32)
        nc.sync.dma_start(ident, ins["ident"])
        aavg = const.tile([128, NT * N], BF16)      # [128, t, n] averaging
        nc.sync.dma_start(aavg, ins["aavgp"])
        mpos = const.tile([NB, M], F32)
        nc.sync.dma_start(mpos, ins["mpos"])
        mneg = const.tile([NB, M], F32)
        nc.sync.dma_start(mneg, ins["mneg"])
        jrow = const.tile([N, 1], F32)
        nc.sync.dma_start(jrow, ins["jrow"])
        selB = const.tile([N, NB], F32)
        nc.sync.dma_start(selB, ins["selB"])
        WB = const.tile([NB, N], F32)
        nc.sync.dma_start(WB, ins["WB"])
        maskbd = const.tile([N, 2 * N], F32)
        nc.sync.dma_start(maskbd, ins["maskbd"])
        seli = const.tile([N, S * NB], F32)
        nc.sync.dma_start(seli, ins["seli"])
        wsel = const.tile([64, NB], F32)
        nc.sync.dma_start(wsel, ins["wsel"])
        ones_n = const.tile([1, N], F32)
        nc.vector.memset(ones_n, 1.0)
        ones192 = const.tile([1, 2 * N], F32)
        nc.vector.memset(ones192, 1.0)
        ones_c = const.tile([NB, 1], F32)
        nc.vector.memset(ones_c, 1.0)
        marg = const.tile([NB, 1], F32)
        nc.vector.memset(marg, MARGIN)

        # ---- feats loads (both layouts, host-staged bf16) ----
        nt_all = pers.tile([128, NT * DPC], BF16)   # natural: [p, (t, d')]
        nc.sync.dma_start(nt_all, ins["natp"])
        ftT = pers.tile([128, S * M], BF16)         # transposed: [p, (kd, m)]
        nc.sync.dma_start(ftT, ins["ftTp"])

        stage = pers.tile([N, PK], F32)             # a2a staging: scores|ctr
        a2a_in = dram.tile([N, PK], F32)
        a2a_out = dram.tile([N, PK], F32)

        # ---- phase 1: centers, norms, partial scores ----
        with tc.tile_pool(name="psumc", bufs=1, space="PSUM") as psumc, \
             tc.tile_pool(name="psums", bufs=1, space="PSUM") as psums, \
             tc.tile_pool(name="psumt", bufs=2, space="PSUM") as psumt, \
             tc.tile_pool(name="psumn", bufs=1, space="PSUM") as psumn, \
             tc.tile_pool(name="workp1", bufs=2) as work:
            ps_c = [psumc.tile([N, 512], F32, name=f"ps_c{h}") for h in range(2)]
            for t in range(NT):
                for h in range(2):
                    nc.tensor.matmul(
                        ps_c[h], lhsT=aavg[:, t * N:(t + 1) * N],
                        rhs=nt_all[:, t * DPC + h * 512: t * DPC + (h + 1) * 512],
                        start=(t == 0), stop=(t == NT - 1),
                    )
            # per-sample squared norms (slice partial), [128, t]
            normcol = pers.tile([128, NT], F32)
            for t in range(NT):
                sqd = work.tile([128, DPC], F32, tag="sqd")
                nc.scalar.activation(
                    sqd, nt_all[:, t * DPC:(t + 1) * DPC], AF.Square,
                    accum_out=normcol[:, t:t + 1],
                )
            # centers psum -> stage cols 1536:2560
            for h in range(2):
                nc.vector.tensor_copy(
                    stage[:, 1536 + h * 512:1536 + (h + 1) * 512], ps_c[h]
                )
            # ctrT2 blocks (x -2, bf16) via PE transpose
            ctrT2 = []
            for kd in range(S):
                tpc = psumt.tile([128, N], F32, tag="tp")
                nc.tensor.transpose(
                    tpc, stage[:, 1536 + kd * 128:1536 + (kd + 1) * 128],
                    ident[:N, :N],
                )
                c2 = work.tile([128, N], BF16, tag=f"ctrT{kd}", bufs=1)
                nc.vector.tensor_scalar_mul(c2, tpc, -2.0)
                ctrT2.append(c2)
            # norm row [1, 1536]
            nrow = pers.tile([1, M], F32)
            for t in range(NT):
                tpn = psumn.tile([1, 128], F32, tag="tpn")
                nc.tensor.transpose(tpn, normcol[:, t:t + 1], ident)
                nc.vector.tensor_copy(nrow[:, t * 128:(t + 1) * 128], tpn)
            # partial scores -> stage cols 0:1536
            s_ps = [psums.tile([N, 512], F32, name=f"s_ps{b}") for b in range(3)]
            for kd in range(S):
                for b in range(3):
                    nc.tensor.matmul(
                        s_ps[b], lhsT=ctrT2[kd],
                        rhs=ftT[:, kd * M + b * 512: kd * M + (b + 1) * 512],
                        start=(kd == 0), stop=False,
                    )
            for b in range(3):
                nc.tensor.matmul(
                    s_ps[b], lhsT=ones_n, rhs=nrow[:, b * 512:(b + 1) * 512],
                    start=False, stop=True,
                )
            for b in range(3):
                nc.vector.tensor_copy(stage[:, b * 512:(b + 1) * 512], s_ps[b])

        nc.sync.dma_start(a2a_in, stage)
        nc.gpsimd.collective_compute(
            "AllToAll", ALU.bypass, replica_groups=RG,
            ins=[a2a_in.opt()], outs=[a2a_out.opt()],
        )

        # prefetch Sqrt/Tanh ACT tables during the collective (depends on
        # stage so it runs after phase-1 Square activations)
        dum2 = pers.tile([1, 8], F32)
        nc.scalar.activation(dum2, stage[0:1, 0:8], AF.Sqrt)
        nc.scalar.activation(dum2, stage[0:1, 0:8], AF.Tanh)

        po = pers.tile([N, PK], F32)                # [(r, b), scores|ctr]
        nc.sync.dma_start(po, a2a_out)

        # ---- mining: fold partials, mask, argmax ----
        offs = []
        with tc.tile_pool(name="psum2", bufs=1, space="PSUM") as psum2, \
             tc.tile_pool(name="psum2b", bufs=2, space="PSUM") as psum2b, \
             tc.tile_pool(name="mine", bufs=1) as mine:
            s12 = psum2.tile([NB, M], F32)
            for b in range(3):
                nc.tensor.matmul(
                    s12[:, b * 512:(b + 1) * 512], lhsT=selB,
                    rhs=po[:, b * 512:(b + 1) * 512], start=True, stop=True,
                )
            pos = mine.tile([NB, M], F32, name="pos")
            neg = mine.tile([NB, M], F32, name="neg")
            nc.vector.tensor_tensor(out=pos, in0=s12, in1=mpos, op=ALU.add)
            nc.vector.tensor_tensor(out=neg, in0=mneg, in1=s12, op=ALU.subtract)
            for iset, sc in enumerate((pos, neg)):
                mx = mine.tile([NB, 8], F32, name=f"mx{iset}")
                nc.vector.max(mx, sc)
                idx = mine.tile([NB, 8], U32, name=f"idx{iset}")
                nc.vector.max_index(idx, mx, sc)
                # widx*8 + j, as u32 offsets into stripe-major lf
                wf = mine.tile([NB, 1], F32, name=f"wf{iset}")
                nc.vector.tensor_copy(wf, idx[:, :1])
                wb96 = psum2b.tile([N, 1], F32, tag="wb")
                nc.tensor.matmul(wb96, lhsT=WB, rhs=wf, start=True, stop=True)
                of = pers.tile([N, 1], F32, name=f"of{iset}")
                nc.vector.tensor_scalar(
                    out=of, in0=wb96, scalar1=8.0, scalar2=jrow,
                    op0=ALU.mult, op1=ALU.add,
                )
                ou = pers.tile([N, 1], U32, name=f"ou{iset}")
                nc.vector.tensor_copy(ou, of)
                offs.append(ou)

        # ---- phase 2: gather winners, cross-product distances ----
        yTall = pers.tile([128, S * 2 * N], F32)    # [l', (kd, set|96)]
        ynrow = pers.tile([1, 2 * N], F32)
        with tc.tile_pool(name="psum3", bufs=2, space="PSUM") as psum3, \
             tc.tile_pool(name="workp2", bufs=1) as work2:
            for iset in range(2):
                y96 = work2.tile([N, DL], F32, name=f"y96_{iset}")
                nc.gpsimd.indirect_dma_start(
                    out=y96, out_offset=None, in_=ins["lf"],
                    in_offset=bass.IndirectOffsetOnAxis(ap=offs[iset][:, :1], axis=0),
                )
                # ynorm via DVE square + ACT Copy-accum (no Square table)
                ysq = work2.tile([N, DL], F32, name=f"ysq{iset}")
                nc.vector.tensor_tensor(out=ysq, in0=y96, in1=y96, op=ALU.mult)
                ycp = work2.tile([N, DL], BF16, name=f"ycp{iset}")
                yno = work2.tile([N, 1], F32, name=f"yno{iset}")
                nc.scalar.activation(ycp, ysq, AF.Copy, accum_out=yno)
                tpy = psum3.tile([1, N], F32, tag="tpy")
                nc.tensor.transpose(tpy, yno, ident[:N, :N])
                nc.vector.tensor_copy(ynrow[:, iset * N:(iset + 1) * N], tpy)
                # yT blocks scaled by -2
                for kd in range(S):
                    tpk = psum3.tile([128, N], F32, tag="tpk")
                    nc.tensor.transpose(
                        tpk, y96[:, kd * 128:(kd + 1) * 128], ident[:N, :N]
                    )
                    nc.vector.tensor_scalar_mul(
                        yTall[:, kd * 2 * N + iset * N: kd * 2 * N + (iset + 1) * N],
                        tpk, -2.0,
                    )
            # xnorms [96, 1] -> row [1, 96]
            xsq = work2.tile([N, DPC], F32, name="xsq")
            nc.vector.tensor_tensor(
                out=xsq, in0=po[:, 1536:PK], in1=po[:, 1536:PK], op=ALU.mult
            )
            xcp = work2.tile([N, DPC], BF16, name="xcp")
            xno = work2.tile([N, 1], F32, name="xno")
            nc.scalar.activation(xcp, xsq, AF.Copy, accum_out=xno)
            tpx = psum3.tile([1, N], F32, tag="tpy")
            nc.tensor.transpose(tpx, xno, ident[:N, :N])
            xnrow = pers.tile([1, N], F32)
            nc.vector.tensor_copy(xnrow, tpx)

        with tc.tile_pool(name="psum4", bufs=1, space="PSUM") as psum4, \
             tc.tile_pool(name="psum5", bufs=2, space="PSUM") as psum5, \
             tc.tile_pool(name="workp3", bufs=1) as work3:
            # xT blocks from po center cols; accumulate cross products
            P = psum4.tile([N, 2 * N], F32)
            xTs = []
            for kd in range(S):
                tpk = psum5.tile([128, N], F32, tag="tpk2")
                nc.tensor.transpose(
                    tpk, po[:, 1536 + kd * 128:1536 + (kd + 1) * 128],
                    ident[:N, :N],
                )
                xT = work3.tile([128, N], F32, name=f"xT{kd}")
                nc.vector.tensor_copy(xT, tpk)
                xTs.append(xT)
            for kd in range(S):
                nc.tensor.matmul(
                    P, lhsT=xTs[kd], rhs=yTall[:, kd * 2 * N:(kd + 1) * 2 * N],
                    start=(kd == 0), stop=False,
                )
            nc.tensor.matmul(P, lhsT=xnrow, rhs=ones192, start=False, stop=False)
            nc.tensor.matmul(P, lhsT=ones_n, rhs=ynrow, start=False, stop=True)

            # d2 -> sqrt -> tanh(d/2) -> mask -> fold over b'
            d2 = work3.tile([N, 2 * N], F32, name="d2")
            nc.vector.tensor_scalar_max(d2, P, 1e-12)
            dsq = work3.tile([N, 2 * N], F32, name="dsq")
            nc.scalar.activation(dsq, d2, AF.Sqrt)
            dtg = work3.tile([N, 2 * N], F32, name="dtg")
            nc.scalar.activation(dtg, dsq, AF.Tanh, scale=0.5)
            dm = work3.tile([N, 2 * N], F32, name="dm")
            nc.vector.tensor_tensor(out=dm, in0=dtg, in1=maskbd, op=ALU.mult)
            # fold 12 col-blocks of 8 (per set): view [96, (set, b', j)]
            v = dm.rearrange("p (s b j) -> p s b j", s=2, j=S)
            f1 = work3.tile([N, 2 * 6 * S], F32, name="f1")
            v1 = f1.rearrange("p (s b j) -> p s b j", s=2, j=S)
            nc.vector.tensor_tensor(
                out=v1, in0=v[:, :, 0:6, :], in1=v[:, :, 6:12, :], op=ALU.add
            )
            f2 = work3.tile([N, 2 * 3 * S], F32, name="f2")
            v2 = f2.rearrange("p (s b j) -> p s b j", s=2, j=S)
            nc.vector.tensor_tensor(
                out=v2, in0=v1[:, :, 0:3, :], in1=v1[:, :, 3:6, :], op=ALU.add
            )
            f3 = work3.tile([N, 2 * S], F32, name="f3")
            nc.vector.tensor_tensor(
                out=f3.rearrange("p (s j) -> p s j", s=2),
                in0=v2[:, :, 0, :], in1=v2[:, :, 1, :], op=ALU.add,
            )
            dsgC = work3.tile([N, 2 * S], F32, name="dsgC")
            nc.vector.tensor_tensor(
                out=dsgC.rearrange("p (s j) -> p s j", s=2),
                in0=f3.rearrange("p (s j) -> p s j", s=2),
                in1=v2[:, :, 2, :], op=ALU.add,
            )

            # fold rows (i, b) -> [12, (i, j)] per set via sel matmuls
            psf = psum5.tile([NB, 2 * 64], F32, tag="psf")
            for s in range(2):
                for i in range(S):
                    nc.tensor.matmul(
                        psf[:, s * 64 + i * S: s * 64 + (i + 1) * S],
                        lhsT=seli[:, i * NB:(i + 1) * NB],
                        rhs=dsgC[:, s * S:(s + 1) * S],
                        start=True, stop=True,
                    )
            # dD grid [64, 81]: cells (i, j) at col 9i + (j+1)
            dD = pers.tile([64, 81], F32)
            nc.vector.memset(dD, 0.0)
            for s in range(2):
                dst = dD[s * 32:s * 32 + NB, 0:81].rearrange(
                    "p (i j) -> p i j", j=9
                )[:, 0:8, 1:9]
                nc.vector.tensor_copy(
                    dst,
                    psf[:, s * 64:(s + 1) * 64].rearrange(
                        "p (i j) -> p i j", j=S
                    ),
                )

        # ---- shortest-path DP on anti-diagonals ----
        dp = pers.tile([64, 81], F32)
        nc.vector.memset(dp, BIGDP)
        nc.vector.memset(dp[:, 1:2], 0.0)
        for kdiag in range(2, 17):
            lo = max(1, kdiag - 8)
            hi = min(8, kdiag - 1)
            cnt = hi - lo + 1
            f0 = 9 * lo + (kdiag - lo)
            t = pers.tile([64, 8], F32, tag="dptmp", bufs=2)
            nc.vector.tensor_tensor(
                out=t[:, :cnt],
                in0=dp[:, f0 - 9:f0 - 9 + 8 * (cnt - 1) + 1:8],
                in1=dp[:, f0 - 1:f0 - 1 + 8 * (cnt - 1) + 1:8],
                op=ALU.min,
            )
            nc.vector.tensor_tensor(
                out=dp[:, f0:f0 + 8 * (cnt - 1) + 1:8],
                in0=t[:, :cnt],
                in1=dD[:, f0 - 9:f0 - 9 + 8 * (cnt - 1) + 1:8],
                op=ALU.add,
            )

        # ---- loss partial ----
        with tc.tile_pool(name="psum6", bufs=1, space="PSUM") as psum6:
            dps = psum6.tile([NB, 1], F32)
            nc.tensor.matmul(dps, lhsT=wsel, rhs=dp[:, 80:81], start=True, stop=True)
            # relu(ap - an + margin) on DVE (avoids a Relu table load)
            r12 = pers.tile([NB, 1], F32)
            nc.vector.tensor_scalar(
                out=r12, in0=dps, scalar1=marg, scalar2=0.0,
                op0=ALU.add, op1=ALU.max,
            )
            lsum = psum6.tile([1, 1], F32)
            nc.tensor.matmul(lsum, lhsT=r12, rhs=ones_c, start=True, stop=True)
            out_sb = pers.tile([1, 1], F32)
            nc.vector.tensor_copy(out_sb, lsum)
        nc.sync.dma_start(out, out_sb)


def build_program():
    nc = bacc.Bacc(
        "TRN2", target_bir_lowering=False, debug=False,
        enable_asserts=False, num_devices=NCORES,
    )
    ins = {
        "natp": nc.dram_tensor("natp", [128, NT * DPC], BF16, kind="ExternalInput").ap(),
        "ftTp": nc.dram_tensor("ftTp", [128, S * M], BF16, kind="ExternalInput").ap(),
        "aavgp": nc.dram_tensor("aavgp", [128, NT * N], BF16, kind="ExternalInput").ap(),
        "mpos": nc.dram_tensor("mpos", [NB, M], F32, kind="ExternalInput").ap(),
        "mneg": nc.dram_tensor("mneg", [NB, M], F32, kind="ExternalInput").ap(),
        "jrow": nc.dram_tensor("jrow", [N, 1], F32, kind="ExternalInput").ap(),
        "selB": nc.dram_tensor("selB", [N, NB], F32, kind="ExternalInput").ap(),
        "WB": nc.dram_tensor("WB", [NB, N], F32, kind="ExternalInput").ap(),
        "maskbd": nc.dram_tensor("maskbd", [N, 2 * N], F32, kind="ExternalInput").ap(),
        "seli": nc.dram_tensor("seli", [N, S * NB], F32, kind="ExternalInput").ap(),
        "wsel": nc.dram_tensor("wsel", [64, NB], F32, kind="ExternalInput").ap(),
        "ident": nc.dram_tensor("ident", [128, 128], F32, kind="ExternalInput").ap(),
        "lf": nc.dram_tensor("lf", [M * S, DL], F32, kind="ExternalInput").ap(),
    }
    out = nc.dram_tensor("out", [1, 1], F32, kind="ExternalOutput").ap()
    with tile.TileContext(nc) as tc:
        build_body(tc, out, ins)
    nc.compile()
    return nc


def make_in_maps(feats, labels, local_features):
    bf16 = mybir.dt.np(BF16)
    feats = np.asarray(feats, dtype=np.float32)
    labf = np.asarray(labels).astype(np.int64)
    anchors = labf[::K]                       # [96] chunk labels
    featsb = feats.astype(bf16)

    lf_flat = np.ascontiguousarray(
        np.asarray(local_features, dtype=np.float32)
        .transpose(0, 2, 1).reshape(M * S, DL)
    )
    ident = np.eye(128, dtype=np.float32)
    # averaging matrix [128, t, n]
    aavg = np.zeros((M, N), dtype=np.float32)
    aavg[np.arange(M), np.arange(M) // K] = 1.0 / K
    aavgp = np.ascontiguousarray(
        aavg.reshape(NT, 128, N).transpose(1, 0, 2).reshape(128, NT * N)
    ).astype(bf16)
    jrowv = np.tile(np.arange(S, dtype=np.float32), NB).reshape(N, 1)
    selB = np.tile(np.eye(NB, dtype=np.float32), (NCORES, 1))      # [96, 12]
    WB = np.kron(np.eye(NB, dtype=np.float32), np.ones((1, S), np.float32))  # [12, 96]
    # maskbd [ (i, b), (s, b', j) ] = delta_{b, b'}
    mb = np.zeros((S, NB, 2, NB, S), dtype=np.float32)
    for b in range(NB):
        mb[:, b, :, b, :] = 1.0
    maskbd = mb.reshape(N, 2 * N)
    # seli [ (i', b), (i, b') ] = delta_{i', i} delta_{b, b'}
    si = np.zeros((S, NB, S, NB), dtype=np.float32)
    for i in range(S):
        for b in range(NB):
            si[i, b, i, b] = 1.0
    seli = si.reshape(N, S * NB)
    wsel = np.zeros((64, NB), dtype=np.float32)
    wsel[0:NB, :] = np.eye(NB)
    wsel[32:32 + NB, :] = -np.eye(NB)

    in_maps = []
    for c in range(NCORES):
        sl = slice(c * DPC, (c + 1) * DPC)
        nat = featsb[:, sl]                                     # [1536, 1024]
        natp = np.ascontiguousarray(
            nat.reshape(NT, 128, DPC).transpose(1, 0, 2).reshape(128, NT * DPC)
        )
        ftTp = np.ascontiguousarray(
            nat.T.reshape(S, 128, M).transpose(1, 0, 2).reshape(128, S * M)
        )
        anc = anchors[c * NB:(c + 1) * NB]                      # [12]
        is_pos = (anc[:, None] == labf[None, :])
        mposc = np.where(is_pos, 0.0, -BIGM).astype(np.float32)
        mnegc = np.where(is_pos, -BIGM, 0.0).astype(np.float32)
        in_maps.append({
            "natp": natp, "ftTp": ftTp, "aavgp": aavgp,
            "mpos": mposc, "mneg": mnegc, "jrow": jrowv,
            "selB": selB, "WB": WB, "maskbd": maskbd, "seli": seli,
            "wsel": wsel, "ident": ident, "lf": lf_flat,
        })
    return in_maps


_NC_CACHE = None


def _get_nc():
    global _NC_CACHE
    if _NC_CACHE is None:
        _NC_CACHE = build_program()
    return _NC_CACHE


def run(feats, labels, local_features, trace=False, **kwargs):
    nc = _get_nc()
    in_maps = make_in_maps(feats, labels, local_features)
    res = bass_utils.run_bass_kernel_spmd(
        nc, in_maps, core_ids=list(range(NCORES)), trace=trace, **kwargs
    )
    partial = sum(float(r["out"][0, 0]) for r in res.results)
    return np.float32(partial / N), res


def kernel(feats, labels, local_features):
    loss, _ = run(feats, labels, local_features)
    return loss


# revision 13
# speedup vs baseline: 1.9018x; 1.9018x over previous
"""Bass/Tile TRN2 kernel for nn_CenterAlignedTripletLoss (8-core SPMD).

Redesign v2 — single-collective feature sharding:
  Each core owns a 1024-wide feature slice of feats (host-staged in both
  natural [m, d] and transposed [d, m] bf16 layouts, so no on-chip DMA
  transposes).  It computes the center slice [96, 1024], per-sample
  partial norms, and partial mining scores s_c[n, m] = ||f_m||^2_c
  - 2 c_n . f_m over its slice.  ONE AllToAll exchanges, per destination
  core r, the partial scores for r's 12 centers plus this core's center
  slice of r's centers (packed [96, 1536+1024] f32).  Each core then
  sums the 8 partial-score blocks (PE fold), mines hardest pos/neg with
  host-staged label masks, indirect-gathers the 24 winning rows in
  stripe-major [96, 1024] layout, and computes the 8x8 stripe distance
  matrices via PE cross-products (instead of elementwise diff-square):
  d2 = ||x||^2 + ||y||^2 - 2 x.y accumulated in one [96, 192] psum.
  sqrt -> tanh(d/2) -> masked block-diag fold -> 9x9 shortest-path DP ->
  relu(ap - an + margin) partial sum.  Host sums the 8 scalars / 96.
"""

import numpy as np
from contextlib import ExitStack

import concourse.bass as bass
import concourse.bacc as bacc
import concourse.tile as tile
from concourse import mybir
from concourse import bass_utils

F32 = mybir.dt.float32
BF16 = mybir.dt.bfloat16
U32 = mybir.dt.uint32
AF = mybir.ActivationFunctionType
ALU = mybir.AluOpType

NCORES = 8
M = 1536          # samples
D = 8192          # feature dim
DPC = D // NCORES # 1024 features per core
N = 96            # centers
NB = N // NCORES  # 12 centers per core
K = 16            # samples per chunk
S = 8             # stripes
DL = 1024         # local feature dim per stripe
MARGIN = 0.3
BIGM = 1.0e9
BIGDP = 1.0e6
RG = [list(range(NCORES))]
NT = M // 128     # 12 natural m-tiles
PK = 1536 + DPC   # packed a2a row width (scores + center slice)


def build_body(tc, out, ins):
    nc = tc.nc

    with ExitStack() as ctx:
        const = ctx.enter_context(tc.tile_pool(name="const", bufs=1))
        pers = ctx.enter_context(tc.tile_pool(name="pers", bufs=1))
        dram = ctx.enter_context(tc.tile_pool(name="dram", bufs=1, space="DRAM"))

        # ---- constants ----
        ident = const.tile([128, 128], F32)
        nc.sync.dma_start(ident, ins["ident"])
        mpos = const.tile([NB, M], F32)
        nc.sync.dma_start(mpos, ins["mpos"])
        mneg = const.tile([NB, M], F32)
        nc.sync.dma_start(mneg, ins["mneg"])
        jrow = const.tile([N, 1], F32)
        nc.sync.dma_start(jrow, ins["jrow"])
        selB = const.tile([N, NB], F32)
        nc.sync.dma_start(selB, ins["selB"])
        WB = const.tile([NB, N], F32)
        nc.sync.dma_start(WB, ins["WB"])
        maskbd = const.tile([N, 2 * N], F32)
        nc.sync.dma_start(maskbd, ins["maskbd"])
        seli = const.tile([N, S * NB], F32)
        nc.sync.dma_start(seli, ins["seli"])
        wsel = const.tile([64, NB], F32)
        nc.sync.dma_start(wsel, ins["wsel"])
        ones_n = const.tile([1, N], F32)
        nc.vector.memset(ones_n, 1.0)
        ones192 = const.tile([1, 2 * N], F32)
        nc.vector.memset(ones192, 1.0)
        ones_c = const.tile([NB, 1], F32)
        nc.vector.memset(ones_c, 1.0)
        marg = const.tile([NB, 1], F32)
        nc.vector.memset(marg, MARGIN)

        # ---- feats load (transposed layout only, host-staged bf16) ----
        ftT = pers.tile([128, S * M], BF16)         # transposed: [p, (kd, m)]
        nc.sync.dma_start(ftT, ins["ftTp"])

        stage = pers.tile([N, PK], F32)             # a2a staging: scores|ctr
        a2a_in = dram.tile([N, PK], F32)
        a2a_out = dram.tile([N, PK], F32)

        # ---- phase 1: centers via free-axis chunk-fold, norms, scores ----
        with tc.tile_pool(name="psums", bufs=1, space="PSUM") as psums, \
             tc.tile_pool(name="psumt", bufs=2, space="PSUM") as psumt, \
             tc.tile_pool(name="psumn", bufs=1, space="PSUM") as psumn, \
             tc.tile_pool(name="workp1", bufs=2) as work:
            # center chunk-sums: [128, (kd, n)] via 4 strided halvings per kd
            ctrF = pers.tile([128, S * N], F32)     # sum over 16 (=16*c_n)
            for kd in range(S):
                h1 = work.tile([128, 768], F32, tag="h1")
                src = ftT[:, kd * M:(kd + 1) * M]
                v0 = src.rearrange("p (a two) -> p a two", two=2)
                nc.vector.tensor_tensor(
                    out=h1, in0=v0[:, :, 0], in1=v0[:, :, 1], op=ALU.add)
                h2 = work.tile([128, 384], F32, tag="h2")
                v1 = h1.rearrange("p (a two) -> p a two", two=2)
                nc.vector.tensor_tensor(
                    out=h2, in0=v1[:, :, 0], in1=v1[:, :, 1], op=ALU.add)
                h3 = work.tile([128, 192], F32, tag="h3")
                v2 = h2.rearrange("p (a two) -> p a two", two=2)
                nc.vector.tensor_tensor(
                    out=h3, in0=v2[:, :, 0], in1=v2[:, :, 1], op=ALU.add)
                v3 = h3.rearrange("p (a two) -> p a two", two=2)
                nc.vector.tensor_tensor(
                    out=ctrF[:, kd * N:(kd + 1) * N],
                    in0=v3[:, :, 0], in1=v3[:, :, 1], op=ALU.add)
            # score lhsT blocks: ctrF * (-2/16) in bf16
            ctrT2 = []
            for kd in range(S):
                c2 = work.tile([128, N], BF16, tag=f"ctrT{kd}", bufs=1)
                nc.vector.tensor_scalar_mul(c2, ctrF[:, kd * N:(kd + 1) * N], -0.125)
                ctrT2.append(c2)
            # stage centers (true scale 1/16) via PE transpose -> cols 1536:
            for kd in range(S):
                tpc = psumt.tile([N, 128], F32, tag="tp")
                nc.tensor.transpose(
                    tpc, ctrF[:, kd * N:(kd + 1) * N], ident
                )
                nc.vector.tensor_scalar_mul(
                    stage[:, 1536 + kd * 128:1536 + (kd + 1) * 128], tpc, 0.0625,
                )
            # per-sample squared norm partials: Square then ones-matmuls
            sq = pers.tile([128, S * M], BF16)
            nc.scalar.activation(sq, ftT, AF.Square)
            ones128 = const.tile([128, 1], BF16)
            nc.vector.memset(ones128, 1.0)
            nrow = pers.tile([1, M], F32)
            for b in range(3):
                nrow_ps = psumn.tile([1, 512], F32, tag="nps")
                for kd in range(S):
                    nc.tensor.matmul(
                        nrow_ps, lhsT=ones128,
                        rhs=sq[:, kd * M + b * 512: kd * M + (b + 1) * 512],
                        start=(kd == 0), stop=(kd == S - 1),
                    )
                nc.vector.tensor_copy(nrow[:, b * 512:(b + 1) * 512], nrow_ps)
            # partial scores -> stage cols 0:1536
            s_ps = [psums.tile([N, 512], F32, name=f"s_ps{b}") for b in range(3)]
            for kd in range(S):
                for b in range(3):
                    nc.tensor.matmul(
                        s_ps[b], lhsT=ctrT2[kd],
                        rhs=ftT[:, kd * M + b * 512: kd * M + (b + 1) * 512],
                        start=(kd == 0), stop=False,
                    )
            for b in range(3):
                nc.tensor.matmul(
                    s_ps[b], lhsT=ones_n, rhs=nrow[:, b * 512:(b + 1) * 512],
                    start=False, stop=True,
                )
            for b in range(3):
                nc.vector.tensor_copy(stage[:, b * 512:(b + 1) * 512], s_ps[b])

        nc.sync.dma_start(a2a_in, stage)
        nc.gpsimd.collective_compute(
            "AllToAll", ALU.bypass, replica_groups=RG,
            ins=[a2a_in.opt()], outs=[a2a_out.opt()],
        )

        # prefetch Sqrt/Tanh ACT tables during the collective (depends on
        # stage so it runs after phase-1 Square activations)
        dum2 = pers.tile([1, 8], F32)
        nc.scalar.activation(dum2, stage[0:1, 0:8], AF.Sqrt)
        nc.scalar.activation(dum2, stage[0:1, 0:8], AF.Tanh)

        po = pers.tile([N, PK], F32)                # [(r, b), scores|ctr]
        nc.sync.dma_start(po, a2a_out)

        # ---- mining: fold partials, mask, argmax ----
        offs = []
        with tc.tile_pool(name="psum2", bufs=1, space="PSUM") as psum2, \
             tc.tile_pool(name="psum2b", bufs=2, space="PSUM") as psum2b, \
             tc.tile_pool(name="mine", bufs=1) as mine:
            s12 = psum2.tile([NB, M], F32)
            for b in range(3):
                nc.tensor.matmul(
                    s12[:, b * 512:(b + 1) * 512], lhsT=selB,
                    rhs=po[:, b * 512:(b + 1) * 512], start=True, stop=True,
                )
            pos = mine.tile([NB, M], F32, name="pos")
            neg = mine.tile([NB, M], F32, name="neg")
            nc.vector.tensor_tensor(out=pos, in0=s12, in1=mpos, op=ALU.add)
            nc.vector.tensor_tensor(out=neg, in0=mneg, in1=s12, op=ALU.subtract)
            for iset, sc in enumerate((pos, neg)):
                mx = mine.tile([NB, 8], F32, name=f"mx{iset}")
                nc.vector.max(mx, sc)
                idx = mine.tile([NB, 8], U32, name=f"idx{iset}")
                nc.vector.max_index(idx, mx, sc)
                # widx*8 + j, as u32 offsets into stripe-major lf
                wf = mine.tile([NB, 1], F32, name=f"wf{iset}")
                nc.vector.tensor_copy(wf, idx[:, :1])
                wb96 = psum2b.tile([N, 1], F32, tag="wb")
                nc.tensor.matmul(wb96, lhsT=WB, rhs=wf, start=True, stop=True)
                of = pers.tile([N, 1], F32, name=f"of{iset}")
                nc.vector.tensor_scalar(
                    out=of, in0=wb96, scalar1=8.0, scalar2=jrow,
                    op0=ALU.mult, op1=ALU.add,
                )
                ou = pers.tile([N, 1], U32, name=f"ou{iset}")
                nc.vector.tensor_copy(ou, of)
                offs.append(ou)

        # ---- phase 2: gather winners, cross-product distances ----
        yTall = pers.tile([128, S * 2 * N], F32)    # [l', (kd, set|96)]
        ynrow = pers.tile([1, 2 * N], F32)
        with tc.tile_pool(name="psum3", bufs=2, space="PSUM") as psum3, \
             tc.tile_pool(name="workp2", bufs=1) as work2:
            for iset in range(2):
                y96 = work2.tile([N, DL], F32, name=f"y96_{iset}")
                nc.gpsimd.indirect_dma_start(
                    out=y96, out_offset=None, in_=ins["lf"],
                    in_offset=bass.IndirectOffsetOnAxis(ap=offs[iset][:, :1], axis=0),
                )
                # ynorm via DVE square + ACT Copy-accum (no Square table)
                ysq = work2.tile([N, DL], F32, name=f"ysq{iset}")
                nc.vector.tensor_tensor(out=ysq, in0=y96, in1=y96, op=ALU.mult)
                ycp = work2.tile([N, DL], BF16, name=f"ycp{iset}")
                yno = work2.tile([N, 1], F32, name=f"yno{iset}")
                nc.scalar.activation(ycp, ysq, AF.Copy, accum_out=yno)
                tpy = psum3.tile([1, N], F32, tag="tpy")
                nc.tensor.transpose(tpy, yno, ident[:N, :N])
                nc.vector.tensor_copy(ynrow[:, iset * N:(iset + 1) * N], tpy)
                # yT blocks scaled by -2
                for kd in range(S):
                    tpk = psum3.tile([128, N], F32, tag="tpk")
                    nc.tensor.transpose(
                        tpk, y96[:, kd * 128:(kd + 1) * 128], ident[:N, :N]
                    )
                    nc.vector.tensor_scalar_mul(
                        yTall[:, kd * 2 * N + iset * N: kd * 2 * N + (iset + 1) * N],
                        tpk, -2.0,
                    )
            # xnorms [96, 1] -> row [1, 96]
            xsq = work2.tile([N, DPC], F32, name="xsq")
            nc.vector.tensor_tensor(
                out=xsq, in0=po[:, 1536:PK], in1=po[:, 1536:PK], op=ALU.mult
            )
            xcp = work2.tile([N, DPC], BF16, name="xcp")
            xno = work2.tile([N, 1], F32, name="xno")
            nc.scalar.activation(xcp, xsq, AF.Copy, accum_out=xno)
            tpx = psum3.tile([1, N], F32, tag="tpy")
            nc.tensor.transpose(tpx, xno, ident[:N, :N])
            xnrow = pers.tile([1, N], F32)
            nc.vector.tensor_copy(xnrow, tpx)

        with tc.tile_pool(name="psum4", bufs=1, space="PSUM") as psum4, \
             tc.tile_pool(name="psum5", bufs=2, space="PSUM") as psum5, \
             tc.tile_pool(name="workp3", bufs=1) as work3:
            # xT blocks from po center cols; accumulate cross products
            P = psum4.tile([N, 2 * N], F32)
            xTs = []
            for kd in range(S):
                tpk = psum5.tile([128, N], F32, tag="tpk2")
                nc.tensor.transpose(
                    tpk, po[:, 1536 + kd * 128:1536 + (kd + 1) * 128],
                    ident[:N, :N],
                )
                xT = work3.tile([128, N], F32, name=f"xT{kd}")
                nc.vector.tensor_copy(xT, tpk)
                xTs.append(xT)
            for kd in range(S):
                nc.tensor.matmul(
                    P, lhsT=xTs[kd], rhs=yTall[:, kd * 2 * N:(kd + 1) * 2 * N],
                    start=(kd == 0), stop=False,
                )
            nc.tensor.matmul(P, lhsT=xnrow, rhs=ones192, start=False, stop=False)
            nc.tensor.matmul(P, lhsT=ones_n, rhs=ynrow, start=False, stop=True)

            # d2 -> sqrt -> tanh(d/2) -> mask -> fold over b'
            d2 = work3.tile([N, 2 * N], F32, name="d2")
            nc.vector.tensor_scalar_max(d2, P, 1e-12)
            dsq = work3.tile([N, 2 * N], F32, name="dsq")
            nc.scalar.activation(dsq, d2, AF.Sqrt)
            dtg = work3.tile([N, 2 * N], F32, name="dtg")
            nc.scalar.activation(dtg, dsq, AF.Tanh, scale=0.5)
            dm = work3.tile([N, 2 * N], F32, name="dm")
            nc.vector.tensor_tensor(out=dm, in0=dtg, in1=maskbd, op=ALU.mult)
            # fold 12 col-blocks of 8 (per set): view [96, (set, b', j)]
            v = dm.rearrange("p (s b j) -> p s b j", s=2, j=S)
            f1 = work3.tile([N, 2 * 6 * S], F32, name="f1")
            v1 = f1.rearrange("p (s b j) -> p s b j", s=2, j=S)
            nc.vector.tensor_tensor(
                out=v1, in0=v[:, :, 0:6, :], in1=v[:, :, 6:12, :], op=ALU.add
            )
            f2 = work3.tile([N, 2 * 3 * S], F32, name="f2")
            v2 = f2.rearrange("p (s b j) -> p s b j", s=2, j=S)
            nc.vector.tensor_tensor(
                out=v2, in0=v1[:, :, 0:3, :], in1=v1[:, :, 3:6, :], op=ALU.add
            )
            f3 = work3.tile([N, 2 * S], F32, name="f3")
            nc.vector.tensor_tensor(
                out=f3.rearrange("p (s j) -> p s j", s=2),
                in0=v2[:, :, 0, :], in1=v2[:, :, 1, :], op=ALU.add,
            )
            dsgC = work3.tile([N, 2 * S], F32, name="dsgC")
            nc.vector.tensor_tensor(
                out=dsgC.rearrange("p (s j) -> p s j", s=2),
                in0=f3.rearrange("p (s j) -> p s j", s=2),
                in1=v2[:, :, 2, :], op=ALU.add,
            )

            # fold rows (i, b) -> [12, (i, j)] per set via sel matmuls
            psf = psum5.tile([NB, 2 * 64], F32, tag="psf")
            for s in range(2):
                for i in range(S):
                    nc.tensor.matmul(
                        psf[:, s * 64 + i * S: s * 64 + (i + 1) * S],
                        lhsT=seli[:, i * NB:(i + 1) * NB],
                        rhs=dsgC[:, s * S:(s + 1) * S],
                        start=True, stop=True,
                    )
            # dD grid [64, 81]: cells (i, j) at col 9i + (j+1)
            dD = pers.tile([64, 81], F32)
            nc.vector.memset(dD, 0.0)
            for s in range(2):
                dst = dD[s * 32:s * 32 + NB, 0:81].rearrange(
                    "p (i j) -> p i j", j=9
                )[:, 0:8, 1:9]
                nc.vector.tensor_copy(
                    dst,
                    psf[:, s * 64:(s + 1) * 64].rearrange(
                        "p (i j) -> p i j", j=S
                    ),
                )

        # ---- shortest-path DP on anti-diagonals ----
        dp = pers.tile([64, 81], F32)
        nc.vector.memset(dp, BIGDP)
        nc.vector.memset(dp[:, 1:2], 0.0)
        for kdiag in range(2, 17):
            lo = max(1, kdiag - 8)
            hi = min(8, kdiag - 1)
            cnt = hi - lo + 1
            f0 = 9 * lo + (kdiag - lo)
            t = pers.tile([64, 8], F32, tag="dptmp", bufs=2)
            nc.vector.tensor_tensor(
                out=t[:, :cnt],
                in0=dp[:, f0 - 9:f0 - 9 + 8 * (cnt - 1) + 1:8],
                in1=dp[:, f0 - 1:f0 - 1 + 8 * (cnt - 1) + 1:8],
                op=ALU.min,
            )
            nc.vector.tensor_tensor(
                out=dp[:, f0:f0 + 8 * (cnt - 1) + 1:8],
                in0=t[:, :cnt],
                in1=dD[:, f0 - 9:f0 - 9 + 8 * (cnt - 1) + 1:8],
                op=ALU.add,
            )

        # ---- loss partial ----
        with tc.tile_pool(name="psum6", bufs=1, space="PSUM") as psum6:
            dps = psum6.tile([NB, 1], F32)
            nc.tensor.matmul(dps, lhsT=wsel, rhs=dp[:, 80:81], start=True, stop=True)
            # relu(ap - an + margin) on DVE (avoids a Relu table load)
            r12 = pers.tile([NB, 1], F32)
            nc.vector.tensor_scalar(
                out=r12, in0=dps, scalar1=marg, scalar2=0.0,
                op0=ALU.add, op1=ALU.max,
            )
            lsum = psum6.tile([1, 1], F32)
            nc.tensor.matmul(lsum, lhsT=r12, rhs=ones_c, start=True, stop=True)
            out_sb = pers.tile([1, 1], F32)
            nc.vector.tensor_copy(out_sb, lsum)
        nc.sync.dma_start(out, out_sb)


def build_program():
    nc = bacc.Bacc(
        "TRN2", target_bir_lowering=False, debug=False,
        enable_asserts=False, num_devices=NCORES,
    )
    ins = {
        "ftTp": nc.dram_tensor("ftTp", [128, S * M], BF16, kind="ExternalInput").ap(),
        "mpos": nc.dram_tensor("mpos", [NB, M], F32, kind="ExternalInput").ap(),
        "mneg": nc.dram_tensor("mneg", [NB, M], F32, kind="ExternalInput").ap(),
        "jrow": nc.dram_tensor("jrow", [N, 1], F32, kind="ExternalInput").ap(),
        "selB": nc.dram_tensor("selB", [N, NB], F32, kind="ExternalInput").ap(),
        "WB": nc.dram_tensor("WB", [NB, N], F32, kind="ExternalInput").ap(),
        "maskbd": nc.dram_tensor("maskbd", [N, 2 * N], F32, kind="ExternalInput").ap(),
        "seli": nc.dram_tensor("seli", [N, S * NB], F32, kind="ExternalInput").ap(),
        "wsel": nc.dram_tensor("wsel", [64, NB], F32, kind="ExternalInput").ap(),
        "ident": nc.dram_tensor("ident", [128, 128], F32, kind="ExternalInput").ap(),
        "lf": nc.dram_tensor("lf", [M * S, DL], BF16, kind="ExternalInput").ap(),
    }
    out = nc.dram_tensor("out", [1, 1], F32, kind="ExternalOutput").ap()
    with tile.TileContext(nc) as tc:
        build_body(tc, out, ins)
    nc.compile()
    return nc


def make_in_maps(feats, labels, local_features):
    bf16 = mybir.dt.np(BF16)
    feats = np.asarray(feats, dtype=np.float32)
    labf = np.asarray(labels).astype(np.int64)
    anchors = labf[::K]                       # [96] chunk labels
    featsb = feats.astype(bf16)

    lf_flat = np.ascontiguousarray(
        np.asarray(local_features, dtype=np.float32)
        .transpose(0, 2, 1).reshape(M * S, DL)
    ).astype(bf16)
    ident = np.eye(128, dtype=np.float32)
    jrowv = np.tile(np.arange(S, dtype=np.float32), NB).reshape(N, 1)
    selB = np.tile(np.eye(NB, dtype=np.float32), (NCORES, 1))      # [96, 12]
    WB = np.kron(np.eye(NB, dtype=np.float32), np.ones((1, S), np.float32))  # [12, 96]
    # maskbd [ (i, b), (s, b', j) ] = delta_{b, b'}
    mb = np.zeros((S, NB, 2, NB, S), dtype=np.float32)
    for b in range(NB):
        mb[:, b, :, b, :] = 1.0
    maskbd = mb.reshape(N, 2 * N)
    # seli [ (i', b), (i, b') ] = delta_{i', i} delta_{b, b'}
    si = np.zeros((S, NB, S, NB), dtype=np.float32)
    for i in range(S):
        for b in range(NB):
            si[i, b, i, b] = 1.0
    seli = si.reshape(N, S * NB)
    wsel = np.zeros((64, NB), dtype=np.float32)
    wsel[0:NB, :] = np.eye(NB)
    wsel[32:32 + NB, :] = -np.eye(NB)

    in_maps = []
    for c in range(NCORES):
        sl = slice(c * DPC, (c + 1) * DPC)
        nat = featsb[:, sl]                                     # [1536, 1024]
        ftTp = np.ascontiguousarray(
            nat.T.reshape(S, 128, M).transpose(1, 0, 2).reshape(128, S * M)
        )
        anc = anchors[c * NB:(c + 1) * NB]                      # [12]
        is_pos = (anc[:, None] == labf[None, :])
        mposc = np.where(is_pos, 0.0, -BIGM).astype(np.float32)
        mnegc = np.where(is_pos, -BIGM, 0.0).astype(np.float32)
        in_maps.append({
            "ftTp": ftTp,
            "mpos": mposc, "mneg": mnegc, "jrow": jrowv,
            "selB": selB, "WB": WB, "maskbd": maskbd, "seli": seli,
            "wsel": wsel, "ident": ident, "lf": lf_flat,
        })
    return in_maps


_NC_CACHE = None


def _get_nc():
    global _NC_CACHE
    if _NC_CACHE is None:
        _NC_CACHE = build_program()
    return _NC_CACHE


def run(feats, labels, local_features, trace=False, **kwargs):
    nc = _get_nc()
    in_maps = make_in_maps(feats, labels, local_features)
    res = bass_utils.run_bass_kernel_spmd(
        nc, in_maps, core_ids=list(range(NCORES)), trace=trace, **kwargs
    )
    partial = sum(float(r["out"][0, 0]) for r in res.results)
    return np.float32(partial / N), res


def kernel(feats, labels, local_features):
    loss, _ = run(feats, labels, local_features)
    return loss
